# revision 3
# baseline (speedup 1.0000x reference)
"""EnhancedMACDCell forward on 8 Trainium2 NeuronCores.

The reference computes, per batch row b of price_series [B, 64]:
    macd[b, j]  = w_fast . price[b, e-12:e] - w_slow . price[b, e-26:e]
                  + (b_fast - b_slow),        e = 64 - 8 + j, j = 0..8
    signal[b]   = w_sig . macd[b, :] + b_sig
    hist[b]     = macd[b, 8] - signal[b]
    out[b]      = tanh(hist[b] * norm_scale + norm_bias)

Everything before the tanh is linear in price_series, so the whole model
collapses to a single 64-tap linear functional per row:
    out[b] = tanh(price[b, :] . u + c0)
with u / c0 computed on the host (float64) from the tiny weight inputs.
Only columns 30..63 of u are nonzero, so per row the device does a
34-wide weighted reduction + tanh over [1M, 64] float32 - purely memory
bound (32 MiB of HBM reads per core).

Device kernel (raw bacc, hand-placed semaphores):
  * x is streamed in full 64-col rows (16 KiB per-partition descriptors
    keep all 16 SDMA engines at line rate; narrower slices explode the
    descriptor count, serialize on descriptor generation, and run below
    engine line rate - measured for 136/392/648/904 B patterns), via
    SWDGE (gpsimd) DMAs that cast f32 -> bf16 in-flight. The cast halves
    the SBUF write traffic and, more importantly, lets the DVE run its
    2x packed perf mode. The two head chunks load as f32 through the two
    HWDGE rings instead, taking 4.2 MB off the SWDGE stream while its Q7
    descriptor path warms up (~1 us).
  * DVE does per-chunk tensor_mul (bf16 x bf16, 2x mode) against a
    replicated weight tile and a f32-accumulating reduce_sum.
  * ACT (scalar) engine builds the replicated weight tile at startup and
    applies tanh(x + c0) in two batched activations.
  * Outputs accumulate in one SBUF tile, flushed by two HWDGE DMAs at
    the very end (mid-stream flushes contend with the saturated load
    stream and slow it down).

Measured on the 8-core spec shape: ~96 us vs 122.8 us for the f32
full-row baseline (load stream ~80 us = 33.6 MB at ~420 GB/s, ~97% of
the 16-engine SDMA fabric ceiling; bf16 keeps the DVE stream (~60 us
busy) off the critical path).

Sharding: pure data parallel - 8 equal batch shards, weights replicated.
"""

import os
import sys
from contextlib import ExitStack

import numpy as np

for _p in ("/opt/trn_rl_repo", "/root/.axon_site/_ro/trn_rl_repo"):
    if os.path.isdir(_p) and _p not in sys.path:
        sys.path.insert(0, _p)

import concourse.bacc as bacc
import concourse.bass as bass
import concourse.mybir as mybir
from concourse import tile
from concourse.bass_utils import run_bass_kernel_spmd

FAST, SLOW, SIG = 12, 26, 9
S = 64
N_CORES = 8
P = 128           # SBUF partitions
C_LO, C_HI = 30, 64
C = C_HI - C_LO   # 34 columns with nonzero weight


def _collapsed_weights(w_fast, b_fast, w_slow, b_slow, w_sig, b_sig,
                       norm_scale, norm_bias):
    """Fold the whole linear pipeline into (u[64], c0)."""
    wf = np.asarray(w_fast, np.float64).reshape(-1)
    ws = np.asarray(w_slow, np.float64).reshape(-1)
    wg = np.asarray(w_sig, np.float64).reshape(-1)
    A = np.zeros((SIG, S), np.float64)
    for j in range(SIG):
        e = S - (SIG - 1) + j
        A[j, e - FAST:e] += wf
        A[j, e - SLOW:e] -= ws
    coeff = -wg.copy()
    coeff[SIG - 1] += 1.0
    u = coeff @ A
    c0 = (float(np.asarray(b_fast).reshape(-1)[0])
          - float(np.asarray(b_slow).reshape(-1)[0])) * coeff.sum() \
        - float(np.asarray(b_sig).reshape(-1)[0])
    ns = float(np.asarray(norm_scale).reshape(-1)[0])
    nb = float(np.asarray(norm_bias).reshape(-1)[0])
    return (u * ns).astype(np.float32), float(c0 * ns + nb)


def _chunks(total_r, r_max, head, tail):
    n_blocks = total_r // r_max
    assert total_r % r_max == 0 and n_blocks >= 2
    assert sum(head) == r_max and sum(tail) == r_max
    chunks = []
    for off, ri in zip(np.cumsum((0,) + head[:-1]), head):
        chunks.append((0, int(off), ri))
    for n in range(1, n_blocks - 1):
        chunks.append((n, 0, r_max))
    for off, ri in zip(np.cumsum((0,) + tail[:-1]), tail):
        chunks.append((n_blocks - 1, int(off), ri))
    return chunks, n_blocks


def _build_bf16(b_core: int, c0: float, bufs: int = 6,
               head=(32, 32), tail=(40, 16, 8), r_max: int = 64) -> bass.Bass:
    nc = bacc.Bacc()
    bf = mybir.dt.bfloat16
    f32 = mybir.dt.float32
    x = nc.declare_dram_parameter("x", [b_core, S], f32, isOutput=False)
    w = nc.declare_dram_parameter("w", [P, C], bf, isOutput=False)
    w32 = nc.declare_dram_parameter("w32", [P, C], f32, isOutput=False)
    y = nc.declare_dram_parameter("y", [b_core], f32, isOutput=True)

    total_r = b_core // P
    chunks, n_blocks = _chunks(total_r, r_max, head, tail)
    n_chunks = len(chunks)
    n_premid = len(head) + (n_blocks - 2)
    nh = len(head)  # f32 HWDGE head chunks
    assert nh == 2

    xb = x[:].rearrange("(n p r) s -> n p r s", p=P, r=r_max)
    yb = y[:].rearrange("(n p r) -> p n r", p=P, r=r_max)
    mid_r = (n_blocks - 1) * r_max
    h_r = max(ri for (_, _, ri) in chunks[:nh])

    with ExitStack() as ctx:
        ef = ctx.enter_context
        xs = [ef(nc.sbuf_tensor(f"xs{k}", [P, r_max * S], bf))
              for k in range(bufs)]
        xh = [ef(nc.sbuf_tensor(f"xh{k}", [P, h_r * S], f32))
              for k in range(nh)]
        pts = [ef(nc.sbuf_tensor(f"pt{k}", [P, r_max * C], bf))
               for k in range(2)]
        pth = ef(nc.sbuf_tensor("pth", [P, h_r * C], f32))
        rt = ef(nc.sbuf_tensor("rt", [P, total_r], f32))
        ot = ef(nc.sbuf_tensor("ot", [P, total_r], f32))
        wt = ef(nc.sbuf_tensor("wt", [P, C], bf))
        wt32 = ef(nc.sbuf_tensor("wt32", [P, C], f32))
        wrep = ef(nc.sbuf_tensor("wrep", [P, r_max * C], bf))
        bt = ef(nc.sbuf_tensor("bt", [P, 1], f32))
        s_slot = [ef(nc.semaphore(f"s_slot{k}")) for k in range(bufs)]
        s_h = [ef(nc.semaphore(f"s_h{k}")) for k in range(nh)]
        s_w = ef(nc.semaphore("s_w"))
        s_w32 = ef(nc.semaphore("s_w32"))
        s_wrep = ef(nc.semaphore("s_wrep"))
        s_red = ef(nc.semaphore("s_red"))
        s_act = ef(nc.semaphore("s_act"))
        s_out = ef(nc.semaphore("s_out"))
        block = ef(nc.Block(no_gpsimd_drain=True))

        def x_dst(i):
            n, off, ri = chunks[i]
            if i < nh:
                slot = xh[i]
            else:
                slot = xs[(i - nh) % bufs]
            return slot[:, :ri * S].rearrange("p (r s) -> p r s", s=S)

        def x_src(i):
            n, off, ri = chunks[i]
            return xb[n][:, off:off + ri, :]

        @block.gpsimd
        def _(gps):
            for i in range(nh, n_chunks):
                k = i - nh
                if i - bufs >= 0:
                    gps.wait_ge(s_red, i - bufs + 1)
                gps.dma_start(x_dst(i), x_src(i)
                              ).then_inc(s_slot[k % bufs], 16)

        @block.sync
        def _(sync):
            sync.dma_start(x_dst(0), x_src(0)).then_inc(s_h[0], 16)
            sync.dma_start(wt32[:, :], w32[:]).then_inc(s_w32, 16)
            sync.dma_start(wt[:, :], w[:]).then_inc(s_w, 16)
            sync.wait_ge(s_act, 1)
            o3 = ot[:, :mid_r].rearrange("p (n r) -> p n r", r=r_max)
            sync.dma_start(yb[:, :n_blocks - 1, :], o3).then_inc(s_out, 16)
            sync.wait_ge(s_act, 2)
            o3t = ot[:, mid_r:].rearrange("p (n r) -> p n r", r=r_max)
            sync.dma_start(yb[:, n_blocks - 1:, :], o3t).then_inc(s_out, 16)
            sync.wait_ge(s_out, 32)

        @block.scalar
        def _(act):
            act.dma_start(x_dst(1), x_src(1)).then_inc(s_h[1], 16)
            act.wait_ge(s_w, 16)
            w3 = wrep[:, :].rearrange("p (r c) -> p r c", c=C)
            nc.scalar.activation(
                w3, wt[:, :].unsqueeze(1).broadcast_to([P, r_max, C]),
                mybir.ActivationFunctionType.Copy,
            ).then_inc(s_wrep, 1)
            act.wait_ge(s_red, n_premid)
            nc.scalar.activation(ot[:, :mid_r], rt[:, :mid_r],
                                 mybir.ActivationFunctionType.Tanh,
                                 bias=bt[:, 0:1], scale=1.0
                                 ).then_inc(s_act, 1)
            act.wait_ge(s_red, n_chunks)
            nc.scalar.activation(ot[:, mid_r:], rt[:, mid_r:],
                                 mybir.ActivationFunctionType.Tanh,
                                 bias=bt[:, 0:1], scale=1.0
                                 ).then_inc(s_act, 1)

        @block.vector
        def _(vec):
            vec.memset(bt[:, :], c0)
            vec.wait_ge(s_w32, 16)
            for i, (n, off, ri) in enumerate(chunks):
                col = n * r_max + off
                if i < nh:
                    vec.wait_ge(s_h[i], 16)
                    x3 = x_dst(i)[:, :, C_LO:C_HI]
                    p3 = pth[:, :ri * C].rearrange("p (r c) -> p r c", c=C)
                    wb = wt32[:, :].unsqueeze(1).broadcast_to([P, ri, C])
                    nc.vector.tensor_mul(p3, x3, wb)
                else:
                    if i == nh:
                        vec.wait_ge(s_wrep, 1)
                    k = i - nh
                    vec.wait_ge(s_slot[k % bufs], 16 * (k // bufs + 1))
                    x3 = x_dst(i)[:, :, C_LO:C_HI]
                    pt = pts[k % 2]
                    p3 = pt[:, :ri * C].rearrange("p (r c) -> p r c", c=C)
                    w3 = wrep[:, :ri * C].rearrange("p (r c) -> p r c", c=C)
                    nc.vector.tensor_mul(p3, x3, w3)
                nc.vector.reduce_sum(rt[:, col:col + ri], p3,
                                     axis=mybir.AxisListType.X
                                     ).then_inc(s_red, 1)

    nc.compile()
    return nc


def _build_generic(b_core: int, c0: float, r: int, bufs: int = 3) -> bass.Bass:
    """Fallback f32 Tile pipeline for shapes the bf16 path can't tile."""
    nc = bacc.Bacc()
    x = nc.declare_dram_parameter("x", [b_core, S], mybir.dt.float32,
                                  isOutput=False)
    w = nc.declare_dram_parameter("w", [P, C], mybir.dt.float32,
                                  isOutput=False)
    y = nc.declare_dram_parameter("y", [b_core], mybir.dt.float32,
                                  isOutput=True)

    rows_per_tile = P * r
    n_tiles = b_core // rows_per_tile
    assert b_core % rows_per_tile == 0

    xv = x[:].rearrange("(n p r) s -> n p r s", p=P, r=r)
    yv = y[:].rearrange("(n p r) -> n p r", p=P, r=r)

    with tile.TileContext(nc) as tc:
        with (
            tc.tile_pool(name="wp", bufs=1) as wp,
            tc.tile_pool(name="xp", bufs=bufs) as xp,
            tc.tile_pool(name="pp", bufs=2) as pp,
            tc.tile_pool(name="rp", bufs=2) as rp,
            tc.tile_pool(name="op", bufs=2) as op,
        ):
            wt = wp.tile([P, C], mybir.dt.float32)
            nc.sync.dma_start(wt[:], w[:])
            bt = wp.tile([P, 1], mybir.dt.float32, tag="bias")
            nc.vector.memset(bt[:], c0)
            for i in range(n_tiles):
                dma_eng = nc.scalar if i % 2 else nc.sync
                xt = xp.tile([P, r * S], mybir.dt.float32)
                x3full = xt[:].rearrange("p (r s) -> p r s", s=S)
                dma_eng.dma_start(x3full, xv[i])
                x3 = x3full[:, :, C_LO:C_HI]
                pt = pp.tile([P, r * C], mybir.dt.float32)
                p3 = pt[:].rearrange("p (r c) -> p r c", c=C)
                wb = wt[:].unsqueeze(1).broadcast_to([P, r, C])
                nc.vector.tensor_mul(p3, x3, wb)
                rt = rp.tile([P, r], mybir.dt.float32)
                nc.vector.reduce_sum(rt[:], p3, axis=mybir.AxisListType.X)
                ot = op.tile([P, r], mybir.dt.float32)
                nc.scalar.activation(ot[:], rt[:],
                                     mybir.ActivationFunctionType.Tanh,
                                     bias=bt[:, 0:1], scale=1.0)
                nc.sync.dma_start(yv[i], ot[:])
    nc.compile()
    return nc


def kernel(**inputs) -> np.ndarray:
    import ml_dtypes

    price = np.ascontiguousarray(np.asarray(inputs["price_series"],
                                            dtype=np.float32))
    B = price.shape[0]
    assert B % N_CORES == 0
    b_core = B // N_CORES

    u, c0 = _collapsed_weights(
        inputs["w_fast"], inputs["b_fast"], inputs["w_slow"],
        inputs["b_slow"], inputs["w_sig"], inputs["b_sig"],
        inputs["norm_scale"], inputs["norm_bias"])

    w32_rep = np.ascontiguousarray(
        np.broadcast_to(u[C_LO:C_HI][None, :], (P, C))).astype(np.float32)
    use_bf16 = b_core % (P * 64) == 0 and b_core // (P * 64) >= 2
    if use_bf16:
        w_rep = w32_rep.astype(ml_dtypes.bfloat16)
        nc = _build_bf16(b_core, c0)
        in_maps = [
            {"x": price[i * b_core:(i + 1) * b_core],
             "w": w_rep, "w32": w32_rep}
            for i in range(N_CORES)
        ]
    else:
        nc = _build_generic(b_core, c0, r=max(1, min(64, b_core // P)))
        in_maps = [
            {"x": price[i * b_core:(i + 1) * b_core], "w": w32_rep}
            for i in range(N_CORES)
        ]
    res = run_bass_kernel_spmd(nc, in_maps, list(range(N_CORES)))
    out = np.concatenate([res.results[i]["y"].reshape(-1)
                          for i in range(N_CORES)])
    return out.reshape(B, 1).astype(np.float32)


# revision 4
# speedup vs baseline: 1.1896x; 1.1896x over previous
"""EnhancedMACDCell forward on 8 Trainium2 NeuronCores.

The reference computes, per batch row b of price_series [B, 64]:
    macd[b, j]  = w_fast . price[b, e-12:e] - w_slow . price[b, e-26:e]
                  + (b_fast - b_slow),        e = 64 - 8 + j, j = 0..8
    signal[b]   = w_sig . macd[b, :] + b_sig
    hist[b]     = macd[b, 8] - signal[b]
    out[b]      = tanh(hist[b] * norm_scale + norm_bias)

Everything before the tanh is linear in price_series, so the whole model
collapses to a single 64-tap linear functional per row:
    out[b] = tanh(price[b, :] . u + c0)
with u / c0 computed on the host (float64) from the tiny weight inputs.
Only columns 30..63 of u are nonzero, so per row the device does a
34-wide weighted reduction + tanh over [1M, 64] float32 - purely memory
bound (32 MiB of HBM reads per core).

Device kernel (raw bacc, hand-placed semaphores):
  * x is streamed in full 64-col rows (16 KiB per-partition descriptors
    keep all 16 SDMA engines at line rate; narrower slices explode the
    descriptor count, serialize on descriptor generation, and run below
    engine line rate - measured for 136/392/648/904 B patterns), via
    SWDGE (gpsimd) DMAs that cast f32 -> bf16 in-flight. The cast halves
    the SBUF write traffic and, more importantly, lets the DVE run its
    2x packed perf mode. The two head chunks load as f32 through the two
    HWDGE rings instead, taking 4.2 MB off the SWDGE stream while its Q7
    descriptor path warms up (~1 us).
  * DVE does per-chunk tensor_mul (bf16 x bf16, 2x mode) against a
    replicated weight tile and a f32-accumulating reduce_sum.
  * ACT (scalar) engine builds the replicated weight tile at startup and
    applies tanh(x + c0) in two batched activations.
  * Outputs accumulate in one SBUF tile, flushed by two HWDGE DMAs at
    the very end (mid-stream flushes contend with the saturated load
    stream and slow it down).

Measured on the 8-core spec shape: ~96 us vs 122.8 us for the f32
full-row baseline (load stream ~80 us = 33.6 MB at ~420 GB/s, ~97% of
the 16-engine SDMA fabric ceiling; bf16 keeps the DVE stream (~60 us
busy) off the critical path).

Sharding: pure data parallel - 8 equal batch shards, weights replicated.
"""

import os
import sys
from contextlib import ExitStack

import numpy as np

for _p in ("/opt/trn_rl_repo", "/root/.axon_site/_ro/trn_rl_repo"):
    if os.path.isdir(_p) and _p not in sys.path:
        sys.path.insert(0, _p)

import concourse.bacc as bacc
import concourse.bass as bass
import concourse.mybir as mybir
from concourse import tile
from concourse.bass_utils import run_bass_kernel_spmd

FAST, SLOW, SIG = 12, 26, 9
S = 64
N_CORES = 8
P = 128           # SBUF partitions
C_LO, C_HI = 30, 64
C = C_HI - C_LO   # 34 columns with nonzero weight


def _collapsed_weights(w_fast, b_fast, w_slow, b_slow, w_sig, b_sig,
                       norm_scale, norm_bias):
    """Fold the whole linear pipeline into (u[64], c0)."""
    wf = np.asarray(w_fast, np.float64).reshape(-1)
    ws = np.asarray(w_slow, np.float64).reshape(-1)
    wg = np.asarray(w_sig, np.float64).reshape(-1)
    A = np.zeros((SIG, S), np.float64)
    for j in range(SIG):
        e = S - (SIG - 1) + j
        A[j, e - FAST:e] += wf
        A[j, e - SLOW:e] -= ws
    coeff = -wg.copy()
    coeff[SIG - 1] += 1.0
    u = coeff @ A
    c0 = (float(np.asarray(b_fast).reshape(-1)[0])
          - float(np.asarray(b_slow).reshape(-1)[0])) * coeff.sum() \
        - float(np.asarray(b_sig).reshape(-1)[0])
    ns = float(np.asarray(norm_scale).reshape(-1)[0])
    nb = float(np.asarray(norm_bias).reshape(-1)[0])
    return (u * ns).astype(np.float32), float(c0 * ns + nb)


def _chunks(total_r, r_max, head, tail):
    n_blocks = total_r // r_max
    assert total_r % r_max == 0 and n_blocks >= 2
    assert sum(head) == r_max and sum(tail) == r_max
    chunks = []
    for off, ri in zip(np.cumsum((0,) + head[:-1]), head):
        chunks.append((0, int(off), ri))
    for n in range(1, n_blocks - 1):
        chunks.append((n, 0, r_max))
    for off, ri in zip(np.cumsum((0,) + tail[:-1]), tail):
        chunks.append((n_blocks - 1, int(off), ri))
    return chunks, n_blocks


def _build_bf16(b_core: int, c0: float, bufs: int = 6,
               head=(32, 32), tail=(40, 16, 8), r_max: int = 64,
               early_tail: int = 0) -> bass.Bass:
    nc = bacc.Bacc()
    bf = mybir.dt.bfloat16
    f32 = mybir.dt.float32
    x = nc.declare_dram_parameter("x", [b_core, S], f32, isOutput=False)
    w = nc.declare_dram_parameter("w", [P, C], bf, isOutput=False)
    w32 = nc.declare_dram_parameter("w32", [P, C], f32, isOutput=False)
    y = nc.declare_dram_parameter("y", [b_core], f32, isOutput=True)

    total_r = b_core // P
    chunks, n_blocks = _chunks(total_r, r_max, head, tail)
    nh = len(head)  # f32 HWDGE head chunks
    assert nh == 2
    if early_tail:
        # move the first `early_tail` tail chunks right after the head so
        # only the small final chunks' compute trails the load stream
        nt = len(tail)
        moved = chunks[-nt:-nt + early_tail]
        chunks = chunks[:nh] + moved + chunks[nh:-nt] + chunks[-nt + early_tail:]
    n_chunks = len(chunks)
    n_premid = n_chunks - (len(tail) - early_tail)

    xb = x[:].rearrange("(n p r) s -> n p r s", p=P, r=r_max)
    yb = y[:].rearrange("(n p r) -> p n r", p=P, r=r_max)
    mid_r = (n_blocks - 1) * r_max
    h_r = max(ri for (_, _, ri) in chunks[:nh])

    with ExitStack() as ctx:
        ef = ctx.enter_context
        xs = [ef(nc.sbuf_tensor(f"xs{k}", [P, r_max * S], bf))
              for k in range(bufs)]
        xh = [ef(nc.sbuf_tensor(f"xh{k}", [P, h_r * S], f32))
              for k in range(nh)]
        pts = [ef(nc.sbuf_tensor(f"pt{k}", [P, r_max * C], bf))
               for k in range(2)]
        pth = ef(nc.sbuf_tensor("pth", [P, h_r * C], f32))
        rt = ef(nc.sbuf_tensor("rt", [P, total_r], f32))
        ot = ef(nc.sbuf_tensor("ot", [P, total_r], f32))
        wt = ef(nc.sbuf_tensor("wt", [P, C], bf))
        wt32 = ef(nc.sbuf_tensor("wt32", [P, C], f32))
        wrep = ef(nc.sbuf_tensor("wrep", [P, r_max * C], bf))
        bt = ef(nc.sbuf_tensor("bt", [P, 1], f32))
        s_slot = [ef(nc.semaphore(f"s_slot{k}")) for k in range(bufs)]
        s_h = [ef(nc.semaphore(f"s_h{k}")) for k in range(nh)]
        s_w = ef(nc.semaphore("s_w"))
        s_w32 = ef(nc.semaphore("s_w32"))
        s_wrep = ef(nc.semaphore("s_wrep"))
        s_red = ef(nc.semaphore("s_red"))
        s_act = ef(nc.semaphore("s_act"))
        s_out = ef(nc.semaphore("s_out"))
        block = ef(nc.Block(no_gpsimd_drain=True))

        def x_dst(i):
            n, off, ri = chunks[i]
            if i < nh:
                slot = xh[i]
            else:
                slot = xs[(i - nh) % bufs]
            return slot[:, :ri * S].rearrange("p (r s) -> p r s", s=S)

        def x_src(i):
            n, off, ri = chunks[i]
            return xb[n][:, off:off + ri, :]

        @block.gpsimd
        def _(gps):
            for i in range(nh, n_chunks):
                k = i - nh
                if i - bufs >= 0:
                    gps.wait_ge(s_red, i - bufs + 1)
                gps.dma_start(x_dst(i), x_src(i)
                              ).then_inc(s_slot[k % bufs], 16)

        @block.sync
        def _(sync):
            sync.dma_start(x_dst(0), x_src(0)).then_inc(s_h[0], 16)
            sync.dma_start(wt32[:, :], w32[:]).then_inc(s_w32, 16)
            sync.dma_start(wt[:, :], w[:]).then_inc(s_w, 16)
            sync.wait_ge(s_act, 1)
            o3 = ot[:, :mid_r].rearrange("p (n r) -> p n r", r=r_max)
            sync.dma_start(yb[:, :n_blocks - 1, :], o3).then_inc(s_out, 16)
            sync.wait_ge(s_act, 2)
            o3t = ot[:, mid_r:].rearrange("p (n r) -> p n r", r=r_max)
            sync.dma_start(yb[:, n_blocks - 1:, :], o3t).then_inc(s_out, 16)
            sync.wait_ge(s_out, 32)

        @block.scalar
        def _(act):
            act.dma_start(x_dst(1), x_src(1)).then_inc(s_h[1], 16)
            act.wait_ge(s_w, 16)
            w3 = wrep[:, :].rearrange("p (r c) -> p r c", c=C)
            nc.scalar.activation(
                w3, wt[:, :].unsqueeze(1).broadcast_to([P, r_max, C]),
                mybir.ActivationFunctionType.Copy,
            ).then_inc(s_wrep, 1)
            act.wait_ge(s_red, n_premid)
            nc.scalar.activation(ot[:, :mid_r], rt[:, :mid_r],
                                 mybir.ActivationFunctionType.Tanh,
                                 bias=bt[:, 0:1], scale=1.0
                                 ).then_inc(s_act, 1)
            act.wait_ge(s_red, n_chunks)
            nc.scalar.activation(ot[:, mid_r:], rt[:, mid_r:],
                                 mybir.ActivationFunctionType.Tanh,
                                 bias=bt[:, 0:1], scale=1.0
                                 ).then_inc(s_act, 1)

        @block.vector
        def _(vec):
            vec.memset(bt[:, :], c0)
            vec.wait_ge(s_w32, 16)
            for i, (n, off, ri) in enumerate(chunks):
                col = n * r_max + off
                if i < nh:
                    vec.wait_ge(s_h[i], 16)
                    x3 = x_dst(i)[:, :, C_LO:C_HI]
                    p3 = pth[:, :ri * C].rearrange("p (r c) -> p r c", c=C)
                    wb = wt32[:, :].unsqueeze(1).broadcast_to([P, ri, C])
                    nc.vector.tensor_mul(p3, x3, wb)
                else:
                    if i == nh:
                        vec.wait_ge(s_wrep, 1)
                    k = i - nh
                    vec.wait_ge(s_slot[k % bufs], 16 * (k // bufs + 1))
                    x3 = x_dst(i)[:, :, C_LO:C_HI]
                    pt = pts[k % 2]
                    p3 = pt[:, :ri * C].rearrange("p (r c) -> p r c", c=C)
                    w3 = wrep[:, :ri * C].rearrange("p (r c) -> p r c", c=C)
                    nc.vector.tensor_mul(p3, x3, w3)
                nc.vector.reduce_sum(rt[:, col:col + ri], p3,
                                     axis=mybir.AxisListType.X
                                     ).then_inc(s_red, 1)

    nc.compile()
    return nc


def _build_generic(b_core: int, c0: float, r: int, bufs: int = 3) -> bass.Bass:
    """Fallback f32 Tile pipeline for shapes the bf16 path can't tile."""
    nc = bacc.Bacc()
    x = nc.declare_dram_parameter("x", [b_core, S], mybir.dt.float32,
                                  isOutput=False)
    w = nc.declare_dram_parameter("w", [P, C], mybir.dt.float32,
                                  isOutput=False)
    y = nc.declare_dram_parameter("y", [b_core], mybir.dt.float32,
                                  isOutput=True)

    rows_per_tile = P * r
    n_tiles = b_core // rows_per_tile
    assert b_core % rows_per_tile == 0

    xv = x[:].rearrange("(n p r) s -> n p r s", p=P, r=r)
    yv = y[:].rearrange("(n p r) -> n p r", p=P, r=r)

    with tile.TileContext(nc) as tc:
        with (
            tc.tile_pool(name="wp", bufs=1) as wp,
            tc.tile_pool(name="xp", bufs=bufs) as xp,
            tc.tile_pool(name="pp", bufs=2) as pp,
            tc.tile_pool(name="rp", bufs=2) as rp,
            tc.tile_pool(name="op", bufs=2) as op,
        ):
            wt = wp.tile([P, C], mybir.dt.float32)
            nc.sync.dma_start(wt[:], w[:])
            bt = wp.tile([P, 1], mybir.dt.float32, tag="bias")
            nc.vector.memset(bt[:], c0)
            for i in range(n_tiles):
                dma_eng = nc.scalar if i % 2 else nc.sync
                xt = xp.tile([P, r * S], mybir.dt.float32)
                x3full = xt[:].rearrange("p (r s) -> p r s", s=S)
                dma_eng.dma_start(x3full, xv[i])
                x3 = x3full[:, :, C_LO:C_HI]
                pt = pp.tile([P, r * C], mybir.dt.float32)
                p3 = pt[:].rearrange("p (r c) -> p r c", c=C)
                wb = wt[:].unsqueeze(1).broadcast_to([P, r, C])
                nc.vector.tensor_mul(p3, x3, wb)
                rt = rp.tile([P, r], mybir.dt.float32)
                nc.vector.reduce_sum(rt[:], p3, axis=mybir.AxisListType.X)
                ot = op.tile([P, r], mybir.dt.float32)
                nc.scalar.activation(ot[:], rt[:],
                                     mybir.ActivationFunctionType.Tanh,
                                     bias=bt[:, 0:1], scale=1.0)
                nc.sync.dma_start(yv[i], ot[:])
    nc.compile()
    return nc


def kernel(**inputs) -> np.ndarray:
    import ml_dtypes

    price = np.ascontiguousarray(np.asarray(inputs["price_series"],
                                            dtype=np.float32))
    B = price.shape[0]
    assert B % N_CORES == 0
    b_core = B // N_CORES

    u, c0 = _collapsed_weights(
        inputs["w_fast"], inputs["b_fast"], inputs["w_slow"],
        inputs["b_slow"], inputs["w_sig"], inputs["b_sig"],
        inputs["norm_scale"], inputs["norm_bias"])

    w32_rep = np.ascontiguousarray(
        np.broadcast_to(u[C_LO:C_HI][None, :], (P, C))).astype(np.float32)
    use_bf16 = b_core % (P * 64) == 0 and b_core // (P * 64) >= 2
    if use_bf16:
        w_rep = w32_rep.astype(ml_dtypes.bfloat16)
        nc = _build_bf16(b_core, c0)
        in_maps = [
            {"x": price[i * b_core:(i + 1) * b_core],
             "w": w_rep, "w32": w32_rep}
            for i in range(N_CORES)
        ]
    else:
        nc = _build_generic(b_core, c0, r=max(1, min(64, b_core // P)))
        in_maps = [
            {"x": price[i * b_core:(i + 1) * b_core], "w": w32_rep}
            for i in range(N_CORES)
        ]
    res = run_bass_kernel_spmd(nc, in_maps, list(range(N_CORES)))
    out = np.concatenate([res.results[i]["y"].reshape(-1)
                          for i in range(N_CORES)])
    return out.reshape(B, 1).astype(np.float32)
